# revision 38
# baseline (speedup 1.0000x reference)
"""Distributed inverse real SHT on 8 Trainium2 NeuronCores (Bass/Tile).

Math (per reference):
    S[c,k,m]  = sum_l x[c,m,l] * pct[m,k,l]          (Legendre synthesis)
    y[c,k,n]  = irfft_{n=1024}(S, norm='forward')
              = sum_m  Sre[c,k,m]*Fc[m,n] + Sim[c,k,m]*Fs[m,n]
    with Fc[m,n] = w_m cos(2*pi*m*n/N), Fs[m,n] = -w_m sin(2*pi*m*n/N),
    w_0 = 1, w_m = 2 otherwise (verified exactly vs np.fft.irfft).
    pct[m,*,l] = 0 for l < m (triangular), and the m=512 row of pct is
    entirely zero (l < 512 always), so the effective mmax is 512.

Sharding: nlat (k) split across the 8 cores -> 64 output latitudes per
core, no inter-core communication.  Each core streams a packed (l >= m)
fp16 slice of pct and x (two-piece DMAs skip the zero rows of short
tiles), does per-m-pair matmuls into PSUM, and stages S^T per 128-m
block.  Blocks are paired into superblocks {0,1} and {2,3}: each
strip's DFT contracts both blocks back-to-back with in-PSUM
accumulation, so the only elementwise accumulation is one PSUM->SBUF
fp16 init copy per strip (superblock 0) and the final E/O fold
(superblock 1, which also folds the fp16 accumulator in via identity
matmuls on the PE).  The S^T staging keeps each extract's free range
disjoint (the tile dependency tracker ignores partition ranges).
"""

import numpy as np
from contextlib import ExitStack


NLAT, NLON = 512, 1024
LMAX, MMAX = 512, 513
M_E = 512            # effective mmax (m=512 row of pct is identically zero)
B, C = 1, 16
NCORES = 8
KC = NLAT // NCORES  # 64 latitudes per core
PAIRS = M_E // 2     # 256 m-pairs
TILE_W = 192         # 128 pct cols (2m x 64k) + 64 x cols (2m x 2ri x 16c)
NBLK = 4             # 128-m blocks


def _plan():
    """One column (192 f-cols x <=128 rows) per (pair,chunk) tile.  Slabs
    cover TWO PSUM banks each, with both banks' K>64 columns leading and
    the short columns trailing, so the slab transfers as at most two
    DMA rectangles (full height, then trimmed height) whose per-partition
    rows stay large enough for efficient DMA descriptors.

    Returns (bank_ops, slab_info, slab_col0, total_cols); bank_ops[G] is
    a list of (pair, l0, K, col) with col the GLOBAL column index;
    slab_info[P] (P = G//2) is (w, w64, h2): the slab's total columns,
    full-height columns, and the height of the trailing short piece."""
    bank_tiles = []
    for G in range(PAIRS // 8):
        tiles = []
        for t in range(8 * G, 8 * G + 8):
            l0 = 2 * t
            L = LMAX - l0
            nch = (L + 127) // 128
            for c in range(nch):
                tiles.append((t, l0 + 128 * c, min(128, L - 128 * c)))
        bank_tiles.append(tiles)

    bank_ops = [None] * len(bank_tiles)
    slab_info = []
    slab_col0 = [0]
    ncols = 0
    for P in range(len(bank_tiles) // 2):
        gA, gB = 2 * P, 2 * P + 1
        merged = [(K, l0, t, g) for g in (gA, gB)
                  for (t, l0, K) in bank_tiles[g]]
        merged.sort(key=lambda x: -x[0])
        per_bank = {gA: [], gB: []}
        for i, (K, l0, t, g) in enumerate(merged):
            per_bank[g].append((t, l0, K, ncols + i))
        for g in (gA, gB):
            bank_ops[g] = sorted(per_bank[g], key=lambda x: -x[2])
        w = len(merged)
        w64 = sum(1 for (K, _, _, _) in merged if K > 64)
        h2 = max([K for (K, _, _, _) in merged if K <= 64], default=0)
        slab_info.append((w, w64, h2))
        ncols += w
        slab_col0.append(ncols)
    return bank_ops, slab_info, slab_col0, ncols


_BANK_OPS, _SLAB_INFO, _SLAB_COL0, NCOLS = _plan()
F_TOT = NCOLS * TILE_W

# Even/odd DFT folding: compute E[n'] = sum_m wc*Re and O[n'] = sum_m ws*Im
# for n' in [0,512) plus the y[512] column (folded into O's zero column 0);
# then y[n'] = E+O, y[1024-n'] = E-O.
NE = NLON // 2       # 512
FW = NE + NE + 16    # wc | ws | (y512 col + pad)


def build_program():
    from concourse import bacc, bass, masks, mybir, tile

    dt = mybir.dt
    nc = bacc.Bacc("TRN2", target_bir_lowering=False, debug=False,
                   num_devices=NCORES)

    stream = nc.dram_tensor("stream", [128, F_TOT], dt.float16,
                            kind="ExternalInput")
    fmat = nc.dram_tensor("fmat", [128, NBLK * FW], dt.float16,
                          kind="ExternalInput")
    y = nc.dram_tensor("y", [C * KC, NLON], dt.float16, kind="ExternalOutput")

    with tile.TileContext(nc) as tc, ExitStack() as ctx:
        sp = ctx.enter_context(tc.tile_pool(name="stream", bufs=5))
        cp = ctx.enter_context(tc.tile_pool(name="const", bufs=1))
        fp = ctx.enter_context(tc.tile_pool(name="fsb", bufs=NBLK))
        ysp = ctx.enter_context(tc.tile_pool(name="ysb", bufs=3))
        snp = ctx.enter_context(tc.tile_pool(name="snat", bufs=2))
        lhp = ctx.enter_context(tc.tile_pool(name="lhs", bufs=20))
        tmp = ctx.enter_context(tc.tile_pool(name="tmp16", bufs=3))
        ps1 = ctx.enter_context(
            tc.tile_pool(name="ps1", bufs=2, space=bass.MemorySpace.PSUM))
        # single PSUM tile; strips ping-pong its two 256-col halves (the
        # dependency tracker keeps the halves independent via free ranges)
        pst = ctx.enter_context(
            tc.tile_pool(name="pst", bufs=1, space=bass.MemorySpace.PSUM))
        # DFT accumulators as single-bank E/O half tiles: five bufs give
        # 2.5 strips of pipeline depth, and the halves free independently
        ps2 = ctx.enter_context(
            tc.tile_pool(name="ps2", bufs=5, space=bass.MemorySpace.PSUM))

        # fp16 output accumulator: partition = (c2,k64) within strip,
        # free = strip*1024 + n  (E in [0:512), O in [512:1024))
        acc = cp.tile([128, 8 * NLON], dt.float16)

        slabs = {}

        def get_slab(si):
            si = min(si, len(_SLAB_INFO) - 1)
            if si not in slabs:
                w, w64, h2 = _SLAB_INFO[si]
                st = sp.tile([128, w * TILE_W], dt.float16, tag="slab")
                o0 = int(_SLAB_COL0[si]) * TILE_W
                # uniform banks need a single DMA; only mixed banks pay a
                # second dispatch for the height-trimmed trailing piece
                if w64 == w:
                    nc.sync.dma_start(
                        out=st[:], in_=stream[:, o0:o0 + w * TILE_W])
                elif w64 == 0:
                    nc.sync.dma_start(
                        out=st[0:h2, :],
                        in_=stream[0:h2, o0:o0 + w * TILE_W])
                else:
                    nc.sync.dma_start(
                        out=st[:, 0:w64 * TILE_W],
                        in_=stream[:, o0:o0 + w64 * TILE_W])
                    nc.sync.dma_start(
                        out=st[0:h2, w64 * TILE_W:w * TILE_W],
                        in_=stream[0:h2,
                                   o0 + w64 * TILE_W:o0 + w * TILE_W])
                slabs[si] = st
            return slabs[si]

        # Process m-blocks smallest-first (block 3 first): the tiny first
        # slabs start compute almost immediately, and each block's carried
        # transpose/DFT work hides inside the next block's larger stream
        # window.  Superblocks (DFT pairs) are {3,2} and {1,0}.
        border = list(range(NBLK - 1, -1, -1))
        bank_seq = [b * 8 + g for b in border for g in range(8)]
        seq_pos = {G: i for i, G in enumerate(bank_seq)}
        pair_seq = [b * 4 + p for b in border for p in range(4)]
        pair_pos = {P: i for i, P in enumerate(pair_seq)}

        # prefetch the first slabs in processing order, then the (tiny)
        # DFT matrices for all four blocks
        get_slab(pair_seq[0])
        get_slab(pair_seq[1])
        get_slab(pair_seq[2])
        fsbs = {}
        for b in border:
            fsbs[b] = fp.tile([128, FW], dt.float16, tag="fsb",
                              name=f"fsb{b}")
            nc.sync.dma_start(
                out=fsbs[b][:], in_=fmat[:, b * FW:(b + 1) * FW])

        ident = cp.tile([128, 128], dt.float16)
        masks.make_identity(nc, ident[:])

        deferred = []   # carried transpose/DFT thunks, drained 2 per bank
        lhs_store = {}  # (block, strip) -> lhs S^T tile awaiting its DFT

        def make_tc_thunk(b, s8, snat_v, ptt):
            """Strip s8 of block b: four PE transposes into a PSUM half,
            then one PSUM->SBUF copy into the strip's lhsT tile."""
            def thunk():
                pt = ptt[:, (s8 % 2) * 256:(s8 % 2) * 256 + 256]
                pt_v = pt.rearrange("p (c r k) -> p c r k", c=2, r=2)
                for ci in range(2):
                    for r in range(2):
                        nc.tensor.transpose(
                            pt_v[:, ci, r, :],
                            snat_v[0:64, r, :, :, :, 2 * s8 + ci],
                            ident[0:64, 0:64])
                # pt free = (c2, ri2, k64) -> lhs free = (ri2, c2, k64)
                lhs = lhp.tile([128, 256], dt.float16, tag="lhs")
                lhs_v = lhs[:].rearrange("p (r c k) -> p r c k", r=2, c=2)
                ceng = nc.vector.tensor_copy if s8 % 2 == 0 else nc.scalar.copy
                ceng(lhs_v[:, :, 0, :], pt_v[:, 0, :, :])
                ceng(lhs_v[:, :, 1, :], pt_v[:, 1, :, :])
                lhs_store[(b, s8)] = lhs
            return thunk

        def make_dft_thunk(sb, bA, bB, s8):
            """Strip s8 of superblock sb: both blocks' DFT matmuls
            accumulate in one PSUM tile; superblock 0 ends with the fp16
            init copy to the accumulator, superblock 1 identity-matmuls
            the accumulator in and folds E/O into the output."""
            def thunk():
                lhsA = lhs_store.pop((bA, s8))
                lhsB = lhs_store.pop((bB, s8))
                fsbA, fsbB = fsbs[bA], fsbs[bB]
                ypE = ps2.tile([128, NE], dt.float32, tag="yp", name="ypE")
                ypO = ps2.tile([128, NE], dt.float32, tag="yp", name="ypO")
                a_sl = acc[:, s8 * NLON:(s8 + 1) * NLON]
                last = sb == 1
                if last:
                    # the accumulator identity-matmuls lead: their inputs
                    # (acc, ident) are ready long before the strip's lhs
                    # copy, so the in-order PE opens each strip with a
                    # guaranteed zero-wait 1024-column run (and one shared
                    # ident weight load) instead of idling into mid pstate
                    nc.tensor.matmul(ypE[:], ident[:],
                                     a_sl[:, 0:NE], start=True, stop=False)
                    nc.tensor.matmul(ypO[:], ident[:],
                                     a_sl[:, NE:2 * NE],
                                     start=True, stop=False)
                # E accumulation group
                nc.tensor.matmul(ypE[:], lhsA[:, 0:128],
                                 fsbA[:, 0:NE], start=not last, stop=False)
                nc.tensor.matmul(ypE[:], lhsB[:, 0:128],
                                 fsbB[:, 0:NE], start=False, stop=True)
                # O + y512 accumulation group
                nc.tensor.matmul(ypO[:], lhsA[:, 128:256],
                                 fsbA[:, NE:2 * NE],
                                 start=not last, stop=False)
                nc.tensor.matmul(ypO[:, 0:8], lhsA[:, 0:128],
                                 fsbA[:, 2 * NE:2 * NE + 8],
                                 start=False, stop=False)
                nc.tensor.matmul(ypO[:], lhsB[:, 128:256],
                                 fsbB[:, NE:2 * NE], start=False, stop=False)
                nc.tensor.matmul(ypO[:, 0:8], lhsB[:, 0:128],
                                 fsbB[:, 2 * NE:2 * NE + 8],
                                 start=False, stop=True)
                if last:
                    # stage both halves in SBUF fp16 (DVE/ACT, freeing the
                    # PSUM slots quickly), then fold on GpSimd (E-half)
                    # and DVE 2x fp16 (mirrored half).
                    t16 = tmp.tile([128, NLON], dt.float16, tag="t16")
                    e16 = t16[:, 0:NE]
                    o16 = t16[:, NE:2 * NE]
                    nc.vector.tensor_copy(e16, ypE[:])
                    nc.scalar.copy(o16, ypO[:])
                    ysb = ysp.tile([128, NLON], dt.float16, tag="ysb")
                    # both folds on DVE (2x fp16 mode, ~0.4 us each beats
                    # GpSimd's 1.4); one combined output DMA per strip
                    # halves the serialized Sync dispatches in the drain
                    nc.vector.tensor_add(
                        ysb[:, 1:NE], e16[:, 1:NE], o16[:, 1:NE])
                    nc.scalar.copy(ysb[:, 0:1], e16[:, 0:1])
                    nc.gpsimd.tensor_copy(ysb[:, NE:NE + 1], o16[:, 0:1])
                    nc.vector.tensor_sub(
                        ysb[:, NE + 1:NLON],
                        e16[:, NE - 1:0:-1],
                        o16[:, NE - 1:0:-1])
                    nc.sync.dma_start(
                        out=y[s8 * 128:(s8 + 1) * 128, :],
                        in_=ysb[:])
                else:
                    ieng = (nc.vector.tensor_copy if s8 % 2 == 0
                            else nc.scalar.copy)
                    ieng(a_sl[:, 0:NE], ypE[:])
                    jeng = (nc.scalar.copy if s8 % 2 == 0
                            else nc.vector.tensor_copy)
                    jeng(a_sl[:, NE:2 * NE], ypO[:])
            return thunk

        for bi, b in enumerate(border):
            # S^T staging for this 128-m block: partition = k (64 rows),
            # free = (ri2, g8, mi2, s8, c16) fp16 -- every extract writes
            # its own contiguous, disjoint 128-wide free quadrant, so the
            # four extracts of a bank run concurrently on DVE and ACT, and
            # for fixed ri the (g, mi, s) dims merge into a single strided
            # free dim as the PE transpose requires.  The transposes thus
            # enumerate m as (g, mi, s); the fmat rows are host-permuted
            # to the same order.
            snat = snp.tile([64, 2 * 8 * 2 * 8 * C], dt.float16, tag="snat")
            snat_v = snat[:].rearrange("p (r g two s c) -> p r g two s c",
                                       r=2, g=8, two=2, s=8, c=C)

            # ---- stage 1: Legendre matmuls, 8 m-pairs per PSUM bank ----
            for g in range(8):
                G = b * 8 + g
                P = G // 2
                st = get_slab(P)
                ppos = pair_pos[P]
                for ahead in (1, 2, 3):
                    if ppos + ahead < len(pair_seq):
                        get_slab(pair_seq[ppos + ahead])
                # drain carried strip work FIRST: its inputs are long
                # ready, so the in-order PE fills the wait for this bank's
                # slab with useful transpose/DFT work.  Draining only one
                # thunk when the queue is short spreads a superblock's DFT
                # drains into the next block's larger stream window.
                for _ in range(2 if len(deferred) > 8 else 1):
                    if deferred:
                        deferred.pop(0)()
                pb = ps1.tile([128, 512], dt.float32, tag="pb")
                pb_v = pb[:].rearrange("p (s mj r c) -> p s mj r c",
                                       s=8, mj=2, r=2, c=C)
                ops = _BANK_OPS[G]
                for j, (t, l0, K, col) in enumerate(ops):
                    o = (col - int(_SLAB_COL0[P])) * TILE_W
                    s = t % 8
                    nc.tensor.matmul(
                        pb[:, s * 64:(s + 1) * 64],
                        st[0:K, o:o + 128],        # (K x [2m x 64k])
                        st[0:K, o + 128:o + 192],  # (K x [2m,2ri,16c])
                        start=(j == 0), stop=(j == len(ops) - 1))
                # extract diagonal (mi==mj) blocks -> snat (cast fp16),
                # split across DVE and ACT
                for mi in range(2):
                    for r in range(2):
                        eng = (nc.vector.tensor_copy if mi == 0
                               else nc.scalar.copy)
                        eng(snat_v[0:64, r, g, mi, :, :],
                            pb_v[mi * 64:(mi + 1) * 64, :, mi, r, :])

            ptt = pst.tile([128, 512], dt.float16, tag="pt", name=f"pt{b}")
            tcs = [make_tc_thunk(b, s8, snat_v, ptt) for s8 in range(8)]
            if bi % 2 == 0:
                deferred += tcs
            else:
                # odd block: its transposes plus the superblock's DFT
                # pairs, staggered two deep so a strip's DFT runs long
                # after its lhs copy -- the in-order PE then streams
                # without semaphore breaks and can ramp to max clock
                sb = bi // 2
                dfts = [make_dft_thunk(sb, border[bi - 1], b, s8)
                        for s8 in range(8)]
                deferred += [tcs[0], tcs[1], tcs[2]]
                for s8 in range(3, 8):
                    deferred += [dfts[s8 - 3], tcs[s8]]
                deferred += [dfts[5], dfts[6], dfts[7]]

        while deferred:
            deferred.pop(0)()

    nc.compile()
    return nc


def _build_fmat():
    m = np.arange(M_E)
    n2 = np.arange(NE)
    w = np.where(m == 0, 1.0, 2.0)
    ang = 2.0 * np.pi * np.outer(m, n2) / NLON
    wc = (w[:, None] * np.cos(ang)).astype(np.float16)     # E weights
    ws = (-w[:, None] * np.sin(ang)).astype(np.float16)    # O weights
    fz = (w * np.where(m % 2 == 0, 1.0, -1.0)).astype(np.float16)  # y[512]
    # The PE transposes enumerate a block's m as (g, mi, s) (bank, pair
    # member, pair-in-bank); permute the DFT rows to match.
    r_t = np.arange(128)
    mloc = (r_t // 16) * 16 + (r_t % 8) * 2 + (r_t % 16) // 8
    fmat = np.zeros((128, NBLK * FW), np.float16)
    for b in range(NBLK):
        rows = b * 128 + mloc
        fmat[:, b * FW:b * FW + NE] = wc[rows]
        fmat[:, b * FW + NE:b * FW + 2 * NE] = ws[rows]
        fmat[:, b * FW + 2 * NE] = fz[rows]
    return fmat


_ALL_OPS = [op for ops in _BANK_OPS for op in ops]


def _pack_streams(x_re, x_im, pct):
    """Per-core packed fp16 stream of shelf-packed (<=128 x 192) tiles."""
    x_re = np.asarray(x_re, np.float32)
    x_im = np.asarray(x_im, np.float32)
    pct = np.asarray(pct, np.float32)

    # x part is core-independent: build once
    template = np.zeros((128, F_TOT), np.float16)
    tv = template.reshape(128, NCOLS, TILE_W)
    for (t, l0, K, col) in _ALL_OPS:
        xr = x_re[0, :, l0:l0 + K, 2 * t:2 * t + 2]   # (c, K, 2m)
        xi = x_im[0, :, l0:l0 + K, 2 * t:2 * t + 2]
        xx = np.stack([xr, xi], axis=0)                # (r, c, K, m)
        tv[0:K, col, 128:] = xx.transpose(2, 3, 0, 1).reshape(K, 64)

    streams = []
    for core in range(NCORES):
        k0 = core * KC
        sbuf = template.copy()
        sv = sbuf.reshape(128, NCOLS, TILE_W)
        for (t, l0, K, col) in _ALL_OPS:
            blk = pct[2 * t:2 * t + 2, k0:k0 + KC, l0:l0 + K]  # (2m, 64k, K)
            sv[0:K, col, 0:128] = blk.transpose(2, 0, 1).reshape(K, 128)
        streams.append(sbuf)
    return streams


_NC_CACHE = [None]


def _get_program():
    if _NC_CACHE[0] is None:
        _NC_CACHE[0] = build_program()
    return _NC_CACHE[0]


def run(x_re, x_im, pct, nlon=NLON, trace=False, trace_kwargs=None):
    from concourse.bass_utils import run_bass_kernel_spmd

    assert int(nlon) == NLON
    nc = _get_program()
    fmat = _build_fmat()
    streams = _pack_streams(x_re, x_im, pct)
    in_maps = [{"stream": streams[i], "fmat": fmat} for i in range(NCORES)]
    res = run_bass_kernel_spmd(nc, in_maps, list(range(NCORES)),
                               trace=trace, **(trace_kwargs or {}))
    out = np.empty((B, C, NLAT, NLON), np.float32)
    for core in range(NCORES):
        yc = res.results[core]["y"].astype(np.float32).reshape(C, KC, NLON)
        out[0, :, core * KC:(core + 1) * KC, :] = yc
    return out, res


def kernel(x_re, x_im, pct, nlon=NLON, **_unused):
    out, _ = run(x_re, x_im, pct, nlon)
    return out


# revision 41
# speedup vs baseline: 1.0020x; 1.0020x over previous
"""Distributed inverse real SHT on 8 Trainium2 NeuronCores (Bass/Tile).

Math (per reference):
    S[c,k,m]  = sum_l x[c,m,l] * pct[m,k,l]          (Legendre synthesis)
    y[c,k,n]  = irfft_{n=1024}(S, norm='forward')
              = sum_m  Sre[c,k,m]*Fc[m,n] + Sim[c,k,m]*Fs[m,n]
    with Fc[m,n] = w_m cos(2*pi*m*n/N), Fs[m,n] = -w_m sin(2*pi*m*n/N),
    w_0 = 1, w_m = 2 otherwise (verified exactly vs np.fft.irfft).
    pct[m,*,l] = 0 for l < m (triangular), and the m=512 row of pct is
    entirely zero (l < 512 always), so the effective mmax is 512.

Sharding: nlat (k) split across the 8 cores -> 64 output latitudes per
core, no inter-core communication.  Each core streams a packed (l >= m)
fp16 slice of pct and x (two-piece DMAs skip the zero rows of short
tiles), does per-m-pair matmuls into PSUM, and stages S^T per 128-m
block.  Blocks are paired into superblocks {0,1} and {2,3}: each
strip's DFT contracts both blocks back-to-back with in-PSUM
accumulation, so the only elementwise accumulation is one PSUM->SBUF
fp16 init copy per strip (superblock 0) and the final E/O fold
(superblock 1, which also folds the fp16 accumulator in via identity
matmuls on the PE).  The S^T staging keeps each extract's free range
disjoint (the tile dependency tracker ignores partition ranges).
"""

import numpy as np
from contextlib import ExitStack


NLAT, NLON = 512, 1024
LMAX, MMAX = 512, 513
M_E = 512            # effective mmax (m=512 row of pct is identically zero)
B, C = 1, 16
NCORES = 8
KC = NLAT // NCORES  # 64 latitudes per core
PAIRS = M_E // 2     # 256 m-pairs
TILE_W = 192         # 128 pct cols (2m x 64k) + 64 x cols (2m x 2ri x 16c)
NBLK = 4             # 128-m blocks


def _plan():
    """One column (192 f-cols x <=128 rows) per (pair,chunk) tile.  Slabs
    cover TWO PSUM banks each, with both banks' K>64 columns leading and
    the short columns trailing, so the slab transfers as at most two
    DMA rectangles (full height, then trimmed height) whose per-partition
    rows stay large enough for efficient DMA descriptors.

    Returns (bank_ops, slab_info, slab_col0, total_cols); bank_ops[G] is
    a list of (pair, l0, K, col) with col the GLOBAL column index;
    slab_info[P] (P = G//2) is (w, w64, h2): the slab's total columns,
    full-height columns, and the height of the trailing short piece."""
    bank_tiles = []
    for G in range(PAIRS // 8):
        tiles = []
        for t in range(8 * G, 8 * G + 8):
            l0 = 2 * t
            L = LMAX - l0
            nch = (L + 127) // 128
            for c in range(nch):
                tiles.append((t, l0 + 128 * c, min(128, L - 128 * c)))
        bank_tiles.append(tiles)

    bank_ops = [None] * len(bank_tiles)
    slab_info = []
    slab_col0 = [0]
    ncols = 0
    for P in range(len(bank_tiles) // 2):
        gA, gB = 2 * P, 2 * P + 1
        merged = [(K, l0, t, g) for g in (gA, gB)
                  for (t, l0, K) in bank_tiles[g]]
        merged.sort(key=lambda x: -x[0])
        per_bank = {gA: [], gB: []}
        for i, (K, l0, t, g) in enumerate(merged):
            per_bank[g].append((t, l0, K, ncols + i))
        for g in (gA, gB):
            bank_ops[g] = sorted(per_bank[g], key=lambda x: -x[2])
        w = len(merged)
        w64 = sum(1 for (K, _, _, _) in merged if K > 64)
        h2 = max([K for (K, _, _, _) in merged if K <= 64], default=0)
        slab_info.append((w, w64, h2))
        ncols += w
        slab_col0.append(ncols)
    return bank_ops, slab_info, slab_col0, ncols


_BANK_OPS, _SLAB_INFO, _SLAB_COL0, NCOLS = _plan()
F_TOT = NCOLS * TILE_W

# Even/odd DFT folding: compute E[n'] = sum_m wc*Re and O[n'] = sum_m ws*Im
# for n' in [0,512) plus the y[512] column (folded into O's zero column 0);
# then y[n'] = E+O, y[1024-n'] = E-O.
NE = NLON // 2       # 512
FW = NE + NE + 16    # wc | ws | (y512 col + pad)


def build_program():
    from concourse import bacc, bass, masks, mybir, tile

    dt = mybir.dt
    nc = bacc.Bacc("TRN2", target_bir_lowering=False, debug=False,
                   num_devices=NCORES)

    stream = nc.dram_tensor("stream", [128, F_TOT], dt.float16,
                            kind="ExternalInput")
    fmat = nc.dram_tensor("fmat", [128, NBLK * FW], dt.float16,
                          kind="ExternalInput")
    y = nc.dram_tensor("y", [C * KC, NLON], dt.float16, kind="ExternalOutput")

    with tile.TileContext(nc) as tc, ExitStack() as ctx:
        sp = ctx.enter_context(tc.tile_pool(name="stream", bufs=5))
        cp = ctx.enter_context(tc.tile_pool(name="const", bufs=1))
        fp = ctx.enter_context(tc.tile_pool(name="fsb", bufs=NBLK))
        ysp = ctx.enter_context(tc.tile_pool(name="ysb", bufs=3))
        snp = ctx.enter_context(tc.tile_pool(name="snat", bufs=2))
        lhp = ctx.enter_context(tc.tile_pool(name="lhs", bufs=20))
        tmp = ctx.enter_context(tc.tile_pool(name="tmp16", bufs=3))
        ps1 = ctx.enter_context(
            tc.tile_pool(name="ps1", bufs=2, space=bass.MemorySpace.PSUM))
        # single PSUM tile; strips ping-pong its two 256-col halves (the
        # dependency tracker keeps the halves independent via free ranges)
        pst = ctx.enter_context(
            tc.tile_pool(name="pst", bufs=1, space=bass.MemorySpace.PSUM))
        # DFT accumulators as single-bank E/O half tiles: five bufs give
        # 2.5 strips of pipeline depth, and the halves free independently
        ps2 = ctx.enter_context(
            tc.tile_pool(name="ps2", bufs=5, space=bass.MemorySpace.PSUM))

        # fp16 output accumulator: partition = (c2,k64) within strip,
        # free = strip*1024 + n  (E in [0:512), O in [512:1024))
        acc = cp.tile([128, 8 * NLON], dt.float16)

        slabs = {}

        def get_slab(si):
            si = min(si, len(_SLAB_INFO) - 1)
            if si not in slabs:
                w, w64, h2 = _SLAB_INFO[si]
                st = sp.tile([128, w * TILE_W], dt.float16, tag="slab")
                o0 = int(_SLAB_COL0[si]) * TILE_W
                # uniform banks need a single DMA; only mixed banks pay a
                # second dispatch for the height-trimmed trailing piece
                if w64 == w:
                    nc.sync.dma_start(
                        out=st[:], in_=stream[:, o0:o0 + w * TILE_W])
                elif w64 == 0:
                    nc.sync.dma_start(
                        out=st[0:h2, :],
                        in_=stream[0:h2, o0:o0 + w * TILE_W])
                else:
                    nc.sync.dma_start(
                        out=st[:, 0:w64 * TILE_W],
                        in_=stream[:, o0:o0 + w64 * TILE_W])
                    nc.sync.dma_start(
                        out=st[0:h2, w64 * TILE_W:w * TILE_W],
                        in_=stream[0:h2,
                                   o0 + w64 * TILE_W:o0 + w * TILE_W])
                slabs[si] = st
            return slabs[si]

        # Process m-blocks smallest-first (block 3 first): the tiny first
        # slabs start compute almost immediately, and each block's carried
        # transpose/DFT work hides inside the next block's larger stream
        # window.  Superblocks (DFT pairs) are {3,2} and {1,0}.
        border = list(range(NBLK - 1, -1, -1))
        bank_seq = [b * 8 + g for b in border for g in range(8)]
        seq_pos = {G: i for i, G in enumerate(bank_seq)}
        pair_seq = [b * 4 + p for b in border for p in range(4)]
        pair_pos = {P: i for i, P in enumerate(pair_seq)}

        # prefetch the first slabs in processing order, then the (tiny)
        # DFT matrices for all four blocks
        get_slab(pair_seq[0])
        get_slab(pair_seq[1])
        get_slab(pair_seq[2])
        fsbs = {}
        for b in border:
            fsbs[b] = fp.tile([128, FW], dt.float16, tag="fsb",
                              name=f"fsb{b}")
            nc.sync.dma_start(
                out=fsbs[b][:], in_=fmat[:, b * FW:(b + 1) * FW])

        ident = cp.tile([128, 128], dt.float16)
        masks.make_identity(nc, ident[:])

        deferred = []   # carried transpose/DFT thunks, drained 2 per bank
        lhs_store = {}  # (block, strip) -> lhs S^T tile awaiting its DFT

        def make_tc_thunk(b, s8, snat_v, ptt):
            """Strip s8 of block b: four PE transposes into a PSUM half,
            then one PSUM->SBUF copy into the strip's lhsT tile."""
            def thunk():
                pt = ptt[:, (s8 % 2) * 256:(s8 % 2) * 256 + 256]
                pt_v = pt.rearrange("p (c r k) -> p c r k", c=2, r=2)
                for ci in range(2):
                    for r in range(2):
                        nc.tensor.transpose(
                            pt_v[:, ci, r, :],
                            snat_v[0:64, r, :, :, :, 2 * s8 + ci],
                            ident[0:64, 0:64])
                # pt free = (c2, ri2, k64) -> lhs free = (ri2, c2, k64)
                lhs = lhp.tile([128, 256], dt.float16, tag="lhs")
                lhs_v = lhs[:].rearrange("p (r c k) -> p r c k", r=2, c=2)
                ceng = nc.vector.tensor_copy if s8 % 2 == 0 else nc.scalar.copy
                ceng(lhs_v[:, :, 0, :], pt_v[:, 0, :, :])
                ceng(lhs_v[:, :, 1, :], pt_v[:, 1, :, :])
                lhs_store[(b, s8)] = lhs
            return thunk

        def make_dft_thunk(sb, bA, bB, s8):
            """Strip s8 of superblock sb: both blocks' DFT matmuls
            accumulate in one PSUM tile; superblock 0 ends with the fp16
            init copy to the accumulator, superblock 1 identity-matmuls
            the accumulator in and folds E/O into the output."""
            def thunk():
                lhsA = lhs_store.pop((bA, s8))
                lhsB = lhs_store.pop((bB, s8))
                fsbA, fsbB = fsbs[bA], fsbs[bB]
                ypE = ps2.tile([128, NE], dt.float32, tag="yp", name="ypE")
                ypO = ps2.tile([128, NE], dt.float32, tag="yp", name="ypO")
                a_sl = acc[:, s8 * NLON:(s8 + 1) * NLON]
                last = sb == 1
                if last:
                    # the accumulator identity-matmuls lead: their inputs
                    # (acc, ident) are ready long before the strip's lhs
                    # copy, so the in-order PE opens each strip with a
                    # guaranteed zero-wait 1024-column run (and one shared
                    # ident weight load) instead of idling into mid pstate
                    nc.tensor.matmul(ypE[:], ident[:],
                                     a_sl[:, 0:NE], start=True, stop=False)
                    nc.tensor.matmul(ypO[:], ident[:],
                                     a_sl[:, NE:2 * NE],
                                     start=True, stop=False)
                # E accumulation group
                nc.tensor.matmul(ypE[:], lhsA[:, 0:128],
                                 fsbA[:, 0:NE], start=not last, stop=False)
                nc.tensor.matmul(ypE[:], lhsB[:, 0:128],
                                 fsbB[:, 0:NE], start=False, stop=True)
                # O + y512 accumulation group
                nc.tensor.matmul(ypO[:], lhsA[:, 128:256],
                                 fsbA[:, NE:2 * NE],
                                 start=not last, stop=False)
                nc.tensor.matmul(ypO[:, 0:8], lhsA[:, 0:128],
                                 fsbA[:, 2 * NE:2 * NE + 8],
                                 start=False, stop=False)
                nc.tensor.matmul(ypO[:], lhsB[:, 128:256],
                                 fsbB[:, NE:2 * NE], start=False, stop=False)
                nc.tensor.matmul(ypO[:, 0:8], lhsB[:, 0:128],
                                 fsbB[:, 2 * NE:2 * NE + 8],
                                 start=False, stop=True)
                if last:
                    # stage both halves in SBUF fp16 (DVE/ACT, freeing the
                    # PSUM slots quickly), then fold on GpSimd (E-half)
                    # and DVE 2x fp16 (mirrored half).
                    t16 = tmp.tile([128, NLON], dt.float16, tag="t16")
                    e16 = t16[:, 0:NE]
                    o16 = t16[:, NE:2 * NE]
                    nc.vector.tensor_copy(e16, ypE[:])
                    nc.scalar.copy(o16, ypO[:])
                    ysb = ysp.tile([128, NLON], dt.float16, tag="ysb")
                    feng = (nc.gpsimd.tensor_add if s8 % 2 == 0
                            else nc.vector.tensor_add)
                    feng(ysb[:, 1:NE], e16[:, 1:NE], o16[:, 1:NE])
                    nc.scalar.copy(ysb[:, 0:1], e16[:, 0:1])
                    nc.sync.dma_start(
                        out=y[s8 * 128:(s8 + 1) * 128, 0:NE],
                        in_=ysb[:, 0:NE])
                    nc.gpsimd.tensor_copy(ysb[:, NE:NE + 1], o16[:, 0:1])
                    nc.vector.tensor_sub(
                        ysb[:, NE + 1:NLON],
                        e16[:, NE - 1:0:-1],
                        o16[:, NE - 1:0:-1])
                    nc.sync.dma_start(
                        out=y[s8 * 128:(s8 + 1) * 128, NE:NLON],
                        in_=ysb[:, NE:NLON])
                else:
                    ieng = (nc.vector.tensor_copy if s8 % 2 == 0
                            else nc.scalar.copy)
                    ieng(a_sl[:, 0:NE], ypE[:])
                    jeng = (nc.scalar.copy if s8 % 2 == 0
                            else nc.vector.tensor_copy)
                    jeng(a_sl[:, NE:2 * NE], ypO[:])
            return thunk

        for bi, b in enumerate(border):
            # S^T staging for this 128-m block: partition = k (64 rows),
            # free = (ri2, g8, mi2, s8, c16) fp16 -- every extract writes
            # its own contiguous, disjoint 128-wide free quadrant, so the
            # four extracts of a bank run concurrently on DVE and ACT, and
            # for fixed ri the (g, mi, s) dims merge into a single strided
            # free dim as the PE transpose requires.  The transposes thus
            # enumerate m as (g, mi, s); the fmat rows are host-permuted
            # to the same order.
            snat = snp.tile([64, 2 * 8 * 2 * 8 * C], dt.float16, tag="snat")
            snat_v = snat[:].rearrange("p (r g two s c) -> p r g two s c",
                                       r=2, g=8, two=2, s=8, c=C)

            # ---- stage 1: Legendre matmuls, 8 m-pairs per PSUM bank ----
            for g in range(8):
                G = b * 8 + g
                P = G // 2
                st = get_slab(P)
                ppos = pair_pos[P]
                for ahead in (1, 2, 3):
                    if ppos + ahead < len(pair_seq):
                        get_slab(pair_seq[ppos + ahead])
                # drain carried strip work FIRST: its inputs are long
                # ready, so the in-order PE fills the wait for this bank's
                # slab with useful transpose/DFT work.  Draining only one
                # thunk when the queue is short spreads a superblock's DFT
                # drains into the next block's larger stream window.
                for _ in range(2 if len(deferred) > 8 else 1):
                    if deferred:
                        deferred.pop(0)()
                pb = ps1.tile([128, 512], dt.float32, tag="pb")
                pb_v = pb[:].rearrange("p (s mj r c) -> p s mj r c",
                                       s=8, mj=2, r=2, c=C)
                ops = _BANK_OPS[G]
                for j, (t, l0, K, col) in enumerate(ops):
                    o = (col - int(_SLAB_COL0[P])) * TILE_W
                    s = t % 8
                    nc.tensor.matmul(
                        pb[:, s * 64:(s + 1) * 64],
                        st[0:K, o:o + 128],        # (K x [2m x 64k])
                        st[0:K, o + 128:o + 192],  # (K x [2m,2ri,16c])
                        start=(j == 0), stop=(j == len(ops) - 1))
                # extract diagonal (mi==mj) blocks -> snat (cast fp16),
                # split across DVE and ACT
                for mi in range(2):
                    for r in range(2):
                        eng = (nc.vector.tensor_copy if mi == 0
                               else nc.scalar.copy)
                        eng(snat_v[0:64, r, g, mi, :, :],
                            pb_v[mi * 64:(mi + 1) * 64, :, mi, r, :])

            ptt = pst.tile([128, 512], dt.float16, tag="pt", name=f"pt{b}")
            tcs = [make_tc_thunk(b, s8, snat_v, ptt) for s8 in range(8)]
            if bi % 2 == 0:
                deferred += tcs
            else:
                # odd block: its transposes plus the superblock's DFT
                # pairs, staggered two deep so a strip's DFT runs long
                # after its lhs copy -- the in-order PE then streams
                # without semaphore breaks and can ramp to max clock
                sb = bi // 2
                dfts = [make_dft_thunk(sb, border[bi - 1], b, s8)
                        for s8 in range(8)]
                deferred += [tcs[0], tcs[1], tcs[2]]
                for s8 in range(3, 8):
                    deferred += [dfts[s8 - 3], tcs[s8]]
                deferred += [dfts[5], dfts[6], dfts[7]]

        while deferred:
            deferred.pop(0)()

    nc.compile()
    return nc


def _build_fmat():
    m = np.arange(M_E)
    n2 = np.arange(NE)
    w = np.where(m == 0, 1.0, 2.0)
    ang = 2.0 * np.pi * np.outer(m, n2) / NLON
    wc = (w[:, None] * np.cos(ang)).astype(np.float16)     # E weights
    ws = (-w[:, None] * np.sin(ang)).astype(np.float16)    # O weights
    fz = (w * np.where(m % 2 == 0, 1.0, -1.0)).astype(np.float16)  # y[512]
    # The PE transposes enumerate a block's m as (g, mi, s) (bank, pair
    # member, pair-in-bank); permute the DFT rows to match.
    r_t = np.arange(128)
    mloc = (r_t // 16) * 16 + (r_t % 8) * 2 + (r_t % 16) // 8
    fmat = np.zeros((128, NBLK * FW), np.float16)
    for b in range(NBLK):
        rows = b * 128 + mloc
        fmat[:, b * FW:b * FW + NE] = wc[rows]
        fmat[:, b * FW + NE:b * FW + 2 * NE] = ws[rows]
        fmat[:, b * FW + 2 * NE] = fz[rows]
    return fmat


_ALL_OPS = [op for ops in _BANK_OPS for op in ops]


def _pack_streams(x_re, x_im, pct):
    """Per-core packed fp16 stream of shelf-packed (<=128 x 192) tiles."""
    x_re = np.asarray(x_re, np.float32)
    x_im = np.asarray(x_im, np.float32)
    pct = np.asarray(pct, np.float32)

    # x part is core-independent: build once
    template = np.zeros((128, F_TOT), np.float16)
    tv = template.reshape(128, NCOLS, TILE_W)
    for (t, l0, K, col) in _ALL_OPS:
        xr = x_re[0, :, l0:l0 + K, 2 * t:2 * t + 2]   # (c, K, 2m)
        xi = x_im[0, :, l0:l0 + K, 2 * t:2 * t + 2]
        xx = np.stack([xr, xi], axis=0)                # (r, c, K, m)
        tv[0:K, col, 128:] = xx.transpose(2, 3, 0, 1).reshape(K, 64)

    streams = []
    for core in range(NCORES):
        k0 = core * KC
        sbuf = template.copy()
        sv = sbuf.reshape(128, NCOLS, TILE_W)
        for (t, l0, K, col) in _ALL_OPS:
            blk = pct[2 * t:2 * t + 2, k0:k0 + KC, l0:l0 + K]  # (2m, 64k, K)
            sv[0:K, col, 0:128] = blk.transpose(2, 0, 1).reshape(K, 128)
        streams.append(sbuf)
    return streams


_NC_CACHE = [None]


def _get_program():
    if _NC_CACHE[0] is None:
        _NC_CACHE[0] = build_program()
    return _NC_CACHE[0]


def run(x_re, x_im, pct, nlon=NLON, trace=False, trace_kwargs=None):
    from concourse.bass_utils import run_bass_kernel_spmd

    assert int(nlon) == NLON
    nc = _get_program()
    fmat = _build_fmat()
    streams = _pack_streams(x_re, x_im, pct)
    in_maps = [{"stream": streams[i], "fmat": fmat} for i in range(NCORES)]
    res = run_bass_kernel_spmd(nc, in_maps, list(range(NCORES)),
                               trace=trace, **(trace_kwargs or {}))
    out = np.empty((B, C, NLAT, NLON), np.float32)
    for core in range(NCORES):
        yc = res.results[core]["y"].astype(np.float32).reshape(C, KC, NLON)
        out[0, :, core * KC:(core + 1) * KC, :] = yc
    return out, res


def kernel(x_re, x_im, pct, nlon=NLON, **_unused):
    out, _ = run(x_re, x_im, pct, nlon)
    return out
